# revision 12
# baseline (speedup 1.0000x reference)
"""MultiHeadAttention TRN2 Bass kernel (fp16 compute, pipelined schedule).

Problem: S=2048, B=2, H=16, d_k=64, D=1024, fp32 in/out.
  q = query @ Wq.T + bq ; k = key @ Wk.T + bk ; v = value @ Wv.T + bv
  score = einsum('qbhd,kbhd->qkbh', q, k) / 8 ; attn = softmax(score, axis=k)
  out = einsum('qkbh,kbhd->qbhd', attn, v) -> reshape -> @ Wo.T + bo

Sharding (8 cores): core c handles batch b = c//4 and heads [4*(c%4), 4*(c%4)+4).
Each core computes its partial output projection (tensor-parallel along the
head dim); the host sums the 4 partials per batch and adds the bias terms
(bv @ Wo.T + bo, the linear-foldable bias contributions).

All matmuls run fp16 (1 cycle/row on the PE; 2x the fp32r rate at 512-wide
moving dim). The ScalarE exp stream (128 x [128,1024], ~136us) is the hard
floor; the schedule aims to hide everything under it:
  - minimal prologue: only K(m0,tb0) + V(q0) + Q(m0,tb0) before attention
  - all other projections drain as fine-grained units between attention steps,
    ordered by the kb-iteration that first needs them (K m1 chunks are only
    needed by the second head-pair, 16 iterations later)
  - softmax normalization + output projection are deferred units too: the PV
    PSUM accumulator is freed immediately by a DVE copy, the rest (gpsimd
    partition-broadcast of the denominator row, DVE reciprocal + multiply,
    out-proj matmuls, DVE evac, DMA) drains under later exps
  - ScalarE does nothing but exp (plus one setup copy); copies/bias-adds run
    on DVE, the denominator broadcast on the otherwise idle GpSimd.
"""

import os

os.environ.setdefault("MYCRO_LOCAL_CACHE", "1")

from collections import deque

import numpy as np

import concourse.bass as bass
import concourse.tile as tile
from concourse import bacc, bass_utils, mybir


def _install_ntff_hook():
    """Provide antenv.axon_hooks when the image lacks it, so trace=True can
    capture NTFF profiles through the axon tunnel. Degrades silently."""
    import contextlib
    import ctypes
    import sys

    if "antenv.axon_hooks" in sys.modules:
        return
    so_path = "/opt/axon/libaxon_pjrt.so"
    if not os.path.exists(so_path):
        return
    try:
        lib = ctypes.CDLL(so_path)
        if not hasattr(lib, "axon_start_nrt_profile"):
            return
        lib.axon_start_nrt_profile.argtypes = [
            ctypes.POINTER(ctypes.c_int64),
            ctypes.c_size_t,
        ]
        lib.axon_start_nrt_profile.restype = ctypes.c_int64
        lib.axon_stop_nrt_profile.argtypes = [ctypes.c_char_p]
        lib.axon_stop_nrt_profile.restype = ctypes.c_int64

        @contextlib.contextmanager
        def _hook(output_dir, device_ids):
            import jax

            jax.devices()
            if device_ids:
                ids = (ctypes.c_int64 * len(device_ids))(*device_ids)
                rc = lib.axon_start_nrt_profile(ids, len(device_ids))
            else:
                rc = lib.axon_start_nrt_profile(None, 0)
            if rc != 0:
                raise RuntimeError(f"axon_start_nrt_profile rc={rc}")
            try:
                yield
            finally:
                n = lib.axon_stop_nrt_profile(str(output_dir).encode())
                print(f"ntff profile: {n} file(s) -> {output_dir}")

        import types

        mod = types.ModuleType("antenv.axon_hooks")
        mod.get_axon_ntff_profile_hook = lambda: _hook
        mod.set_axon_ntff_profile_hook = lambda h: None
        sys.modules["antenv.axon_hooks"] = mod
    except Exception:
        pass


_install_ntff_hook()

F32 = mybir.dt.float32
F16 = mybir.dt.float16
AF = mybir.ActivationFunctionType

S = 2048          # sequence length
B = 2             # batch
H = 16            # total heads
DK = 64           # head dim
D = 1024          # model dim
NCORES = 8
HL = H // (NCORES // B)   # heads per core = 4
HC = HL * DK              # head cols per core = 256
T = S                     # tokens per core (one batch element)
P = 128
QB = 512                  # q block (matmul free dim)
NKB = T // P              # 16 k blocks
NQB = T // QB             # 4 q blocks
NKC = D // P              # 8 contraction chunks for projections
VW = DK + 1               # 65: head value cols + ones column
EXP_BIAS = -2.0           # exp range shift; cancels in softmax normalization


def build_module():
    nc = bacc.Bacc("TRN2", target_bir_lowering=False, debug=False)

    xqT = nc.dram_tensor("xqT", [D, T], F16, kind="ExternalInput").ap()
    xkT = nc.dram_tensor("xkT", [D, T], F16, kind="ExternalInput").ap()
    xvT = nc.dram_tensor("xvT", [D, T], F16, kind="ExternalInput").ap()
    # weights pre-chunked on host to [128, kc/c, cols] partition-major layout
    wqC = nc.dram_tensor("wqC", [P, NKC * HC], F16, kind="ExternalInput").ap()
    wkC = nc.dram_tensor("wkC", [P, NKC * HC], F16, kind="ExternalInput").ap()
    wvC = nc.dram_tensor("wvC", [P, NKC * HC], F16, kind="ExternalInput").ap()
    woC = nc.dram_tensor("woC", [P, 2 * D], F16, kind="ExternalInput").ap()
    bqv = nc.dram_tensor("bqv", [HC], F32, kind="ExternalInput").ap()
    bkv = nc.dram_tensor("bkv", [HC], F32, kind="ExternalInput").ap()
    out = nc.dram_tensor("out", [T, D], F32, kind="ExternalOutput").ap()

    with tile.TileContext(nc) as tc:
        kernel_body(tc, xqT, xkT, xvT, wqC, wkC, wvC, woC, bqv, bkv, out)

    nc.compile()
    return nc


def kernel_body(tc, xqT, xkT, xvT, wqC, wkC, wvC, woC, bqv, bkv, out):
    nc = tc.nc

    with (
        tc.tile_pool(name="consts", bufs=1) as consts,
        tc.tile_pool(name="xk_p", bufs=1) as xk_p,
        tc.tile_pool(name="xq_p", bufs=1) as xq_p,
        tc.tile_pool(name="xv_p", bufs=10) as xv_p,
        tc.tile_pool(name="persist", bufs=1) as persist,
        tc.tile_pool(name="attn", bufs=6) as attn_pool,
        tc.tile_pool(name="small", bufs=4) as small,
        tc.tile_pool(name="outs", bufs=3) as outs,
        tc.tile_pool(name="ps_mm", bufs=2, space="PSUM") as ps_mm,
        tc.tile_pool(name="ps_sc", bufs=2, space="PSUM") as ps_sc,
        tc.tile_pool(name="ps_pv", bufs=2, space="PSUM") as ps_pv,
    ):
        # ---------------- constants (DMA-ordered: K weights first) ----------
        wk_s = consts.tile([P, NKC, HC], F16)
        nc.sync.dma_start(wk_s, wkC.rearrange("p (kc m) -> p kc m", kc=NKC))
        bk_s = consts.tile([P, HC // P], F32)
        nc.sync.dma_start(bk_s, bkv.rearrange("(m p) -> p m", p=P))

        ones16 = consts.tile([P, DK], F16)
        nc.vector.memset(ones16, 1.0)
        expb = consts.tile([P, 1], F32)
        nc.vector.memset(expb, EXP_BIAS)

        # ---------------- persistent activations ----------------
        QT = [persist.tile([P, T], F16, name=f"QT{m}") for m in range(2)]
        KT = [persist.tile([P, T], F16, name=f"KT{m}") for m in range(2)]
        V = persist.tile([P, NKB, HL * VW], F16, name="V")
        AC = [persist.tile([P, T], F16, name=f"AC{c}") for c in range(2)]

        # ones columns of V (denominator trick); one cheap ScalarE copy
        nc.scalar.activation(
            V.rearrange("p t (h c) -> p t h c", c=VW)[:, :, :, DK],
            ones16[:, : NKB * HL].rearrange("p (t h) -> p t h", h=HL),
            AF.Copy,
        )

        # x quarter tiles [128, 512].  xk/xq quarters stay resident (both m
        # chunks need them at different times); xv quarters rotate.
        xkt = {}  # (kc, tb) -> tile
        xqt = {}
        _xid = [0]

        def x_load(pool, store, xT, kc, tb, tag):
            _xid[0] += 1
            t = pool.tile([P, QB], F16, tag=f"{tag}{kc}_{tb}" if store is not None else "xv",
                          name=f"{tag}_{_xid[0]}")
            nc.sync.dma_start(t, xT[kc * P : (kc + 1) * P, tb * QB : (tb + 1) * QB])
            if store is not None:
                store[kc, tb] = t
            return t

        # ---------------- projection unit builders ----------------
        def qk_loads(store, pool, xT, tb, tag):
            return [
                (lambda kc=kc: x_load(pool, store, xT, kc, tb, tag))
                for kc in range(NKC)
            ]

        def qk_job(store, w_s, b_s, dst, tag, tb, m):
            # one (tb, m) projection job: psum alloc + 8 matmuls + bias evac
            stj = {}
            units = []

            def mk_start(stj=stj):
                stj["ps"] = ps_mm.tile([P, QB], F32, tag="mm", name=f"pj_{tag}{m}{tb}")

            units.append(mk_start)
            for kc in range(NKC):

                def mk_mm(kc=kc, stj=stj):
                    nc.tensor.matmul(
                        stj["ps"],
                        lhsT=w_s[:, kc, m * P : (m + 1) * P],
                        rhs=store[kc, tb],
                        start=(kc == 0),
                        stop=(kc == NKC - 1),
                    )

                units.append(mk_mm)

            def mk_evac(stj=stj):
                nc.vector.tensor_scalar_add(
                    dst[m][:, tb * QB : (tb + 1) * QB], stj["ps"], b_s[:, m : m + 1]
                )

            units.append(mk_evac)
            return units

        def v_units(tb):
            # V projection for quarter tb: 8 x loads + 4 token blocks,
            # each block split into two units (psum+4mm / 4mm+evac).
            units = []
            st = {}
            for kc in range(NKC):

                def mk_load(kc=kc, st=st):
                    st[kc] = x_load(xv_p, None, xvT, kc, tb, "xv")

                units.append(mk_load)
            for i in range(QB // P):
                t128 = tb * (QB // P) + i
                stj = {}

                def mk_a(i=i, stj=stj, st=st, t128=t128):
                    stj["ps"] = ps_mm.tile([P, HC], F32, tag="mm", name=f"pv_{t128}")
                    for kc in range(4):
                        nc.tensor.matmul(
                            stj["ps"],
                            lhsT=st[kc][:, i * P : (i + 1) * P],
                            rhs=wv_s[:, kc, :],
                            start=(kc == 0),
                            stop=False,
                        )

                def mk_b(i=i, stj=stj, st=st, t128=t128):
                    for kc in range(4, NKC):
                        nc.tensor.matmul(
                            stj["ps"],
                            lhsT=st[kc][:, i * P : (i + 1) * P],
                            rhs=wv_s[:, kc, :],
                            start=False,
                            stop=(kc == NKC - 1),
                        )
                    nc.vector.tensor_copy(
                        V[:, t128].rearrange("p (h c) -> p h c", c=VW)[:, :, :DK],
                        stj["ps"].rearrange("p (h c) -> p h c", c=DK),
                    )

                units.append(mk_a)
                units.append(mk_b)
            return units

        # ---------------- stage A: minimal prologue ----------------
        for u in qk_loads(xkt, xk_p, xkT, 0, "xk"):
            u()
        for u in qk_job(xkt, wk_s, bk_s, KT, "xk", 0, 0):
            u()

        wv_s = consts.tile([P, NKC, HC], F16)
        nc.sync.dma_start(wv_s, wvC.rearrange("p (kc m) -> p kc m", kc=NKC))
        for u in v_units(0):
            u()

        wq_s = consts.tile([P, NKC, HC], F16)
        nc.sync.dma_start(wq_s, wqC.rearrange("p (kc m) -> p kc m", kc=NKC))
        bq_s = consts.tile([P, HC // P], F32)
        nc.sync.dma_start(bq_s, bqv.rearrange("(m p) -> p m", p=P))
        for u in qk_loads(xqt, xq_p, xqT, 0, "xq"):
            u()
        for u in qk_job(xqt, wq_s, bq_s, QT, "xq", 0, 0):
            u()

        wo_s = consts.tile([P, HC // P, D], F16)
        nc.sync.dma_start(wo_s, woC.rearrange("p (c n) -> p c n", c=2))

        # ---------------- zip queue: remaining projections ------------------
        zq = deque()
        for tb in (1, 2, 3):
            zq.extend(qk_loads(xkt, xk_p, xkT, tb, "xk"))
            zq.extend(qk_job(xkt, wk_s, bk_s, KT, "xk", tb, 0))
            zq.extend(v_units(tb))
        zq.extend(qk_job(xqt, wq_s, bq_s, QT, "xq", 0, 1))
        for tb in (0, 1, 2, 3):
            zq.extend(qk_job(xkt, wk_s, bk_s, KT, "xk", tb, 1))
        for tb in (1, 2, 3):
            zq.extend(qk_loads(xqt, xq_p, xqT, tb, "xq"))
            zq.extend(qk_job(xqt, wq_s, bq_s, QT, "xq", tb, 0))
            zq.extend(qk_job(xqt, wq_s, bq_s, QT, "xq", tb, 1))

        urgent = deque()  # deferred normalize / out-proj units (FIFO)

        def drain(n):
            for _ in range(n):
                if urgent:
                    urgent.popleft()()
                elif zq:
                    zq.popleft()()

        # ---------------- deferred normalize / output projection ------------
        def norm_units(qb, hp, pv0, pv1):
            # The pv-PSUM evacuation copies are emitted SYNCHRONOUSLY here
            # (the next head-pair's pv tiles re-bind these PSUM buffers at
            # emission time, so a deferred reader would race).  Only the
            # SBUF-side normalization work is deferred.
            m = hp
            st = {}
            for h, pv in ((2 * hp, pv0), (2 * hp + 1, pv1)):
                pvs = small.tile([VW, QB], F32, tag="pvs", name=f"pvs_{qb}_{h}")
                nc.vector.tensor_copy(pvs, pv)
                st[h] = pvs
                # denominator row re-homed to partition 0 (fp16) for the
                # PE outer-product broadcast
                dn = small.tile([1, QB], F16, tag="dn", name=f"dn_{qb}_{h}")
                nc.vector.tensor_copy(dn, pv[DK : DK + 1, :])
                st[h, "d"] = dn

            def bc_rcp(h, st=st):
                db = ps_mm.tile([DK, QB], F32, tag="mm", name=f"db_{qb}_{h}")
                nc.tensor.matmul(
                    db, lhsT=ones16[0:1, :], rhs=st[h, "d"], start=True, stop=True
                )
                rb = small.tile([DK, QB], F32, tag="rcp", name=f"rb_{qb}_{h}")
                nc.vector.reciprocal_approx_fast(rb, db)
                st[h, "r"] = rb

            def mul(h, st=st):
                off = 64 * (h % 2)
                nc.vector.tensor_mul(
                    AC[m][off : off + DK, qb * QB : (qb + 1) * QB],
                    st[h][:DK, :],
                    st[h, "r"],
                )

            h0, h1 = 2 * hp, 2 * hp + 1
            return [
                lambda: bc_rcp(h0),
                lambda: mul(h0),
                lambda: bc_rcp(h1),
                lambda: mul(h1),
            ]

        def oproj_units(qb):
            units = []
            for i in range(QB // P):
                t128 = qb * (QB // P) + i
                stj = {}

                def mk_mm(t128=t128, stj=stj):
                    stj["ps"] = [
                        ps_mm.tile([P, 512], F32, tag="mm", name=f"po_{t128}{n}")
                        for n in range(2)
                    ]
                    for c in range(2):
                        for n in range(2):
                            nc.tensor.matmul(
                                stj["ps"][n],
                                lhsT=AC[c][:, t128 * P : (t128 + 1) * P],
                                rhs=wo_s[:, c, n * 512 : (n + 1) * 512],
                                start=(c == 0),
                                stop=(c == 1),
                            )

                def mk_out(t128=t128, stj=stj):
                    ob = outs.tile([P, D], F32, tag="ob", name=f"ob_{t128}")
                    for n in range(2):
                        nc.vector.tensor_copy(ob[:, n * 512 : (n + 1) * 512],
                                              stj["ps"][n])
                    nc.sync.dma_start(out[t128 * P : (t128 + 1) * P, :], ob)

                units.append(mk_mm)
                units.append(mk_out)
            return units

        # ---------------- attention ----------------
        it = [0]

        def drain_n():
            i = it[0]
            it[0] += 1
            return 10 if i < 16 else (5 if i < 32 else 2)

        for qb in range(NQB):
            for hp in range(2):
                m = hp  # heads (2*hp, 2*hp+1) live in QT/KT chunk m
                h0, h1 = 2 * hp, 2 * hp + 1
                pv0 = ps_pv.tile([VW, QB], F32, tag="pv", name=f"pv_{qb}_{h0}")
                pv1 = ps_pv.tile([VW, QB], F32, tag="pv", name=f"pv_{qb}_{h1}")

                def emit_pv(kb, at, pv0=pv0, pv1=pv1, h0=h0, h1=h1):
                    nc.tensor.matmul(
                        pv0,
                        lhsT=V[:, kb, VW * h0 : VW * (h0 + 1)],
                        rhs=at[:, :QB],
                        start=(kb == 0),
                        stop=(kb == NKB - 1),
                    )
                    nc.tensor.matmul(
                        pv1,
                        lhsT=V[:, kb, VW * h1 : VW * (h1 + 1)],
                        rhs=at[:, QB:],
                        start=(kb == 0),
                        stop=(kb == NKB - 1),
                    )

                # Software-pipelined: PV for block kb issues after the score
                # pair for kb+1, giving the exp a full score-pair of slack.
                prev = None
                for kb in range(NKB):
                    sc = ps_sc.tile(
                        [P, 2 * QB], F32, tag="sc", name=f"sc_{qb}_{hp}_{kb}"
                    )
                    nc.tensor.matmul(
                        sc[:, :QB],
                        lhsT=KT[m][0:DK, kb * P : (kb + 1) * P],
                        rhs=QT[m][0:DK, qb * QB : (qb + 1) * QB],
                        start=True,
                        stop=True,
                    )
                    nc.tensor.matmul(
                        sc[:, QB:],
                        lhsT=KT[m][DK:P, kb * P : (kb + 1) * P],
                        rhs=QT[m][DK:P, qb * QB : (qb + 1) * QB],
                        start=True,
                        stop=True,
                    )
                    at = attn_pool.tile(
                        [P, 2 * QB], F16, tag="at", name=f"at_{qb}_{hp}_{kb}"
                    )
                    nc.scalar.activation(at, sc, AF.Exp, scale=0.125, bias=expb)
                    if prev is not None:
                        emit_pv(*prev)
                    prev = (kb, at)
                    drain(drain_n())
                emit_pv(*prev)

                urgent.extend(norm_units(qb, hp, pv0, pv1))
            urgent.extend(oproj_units(qb))

        while urgent or zq:
            drain(8)


_module_cache = None


def get_module():
    global _module_cache
    if _module_cache is None:
        _module_cache = build_module()
    return _module_cache


def _chunk_w(wT):
    # [D, HC] -> [128, NKC*HC]: partition-major kc blocks, contiguous rows
    return np.ascontiguousarray(
        wT.reshape(NKC, P, HC).transpose(1, 0, 2).reshape(P, NKC * HC)
    )


def shard_inputs(query, key, value, Wq, bq, Wk, bk, Wv, bv, Wo, bo):
    """Build the 8 per-core input maps (host-side layout/dtype transforms)."""
    f = np.float32
    h = np.float16
    xT = {}
    for b in range(B):
        xT["q", b] = np.ascontiguousarray(np.asarray(query)[:, b, :].T.astype(h))
        xT["k", b] = np.ascontiguousarray(np.asarray(key)[:, b, :].T.astype(h))
        xT["v", b] = np.ascontiguousarray(np.asarray(value)[:, b, :].T.astype(h))
    Wq, Wk, Wv, Wo = (np.asarray(w, f) for w in (Wq, Wk, Wv, Wo))
    bq, bk = np.asarray(bq, f), np.asarray(bk, f)
    in_maps = []
    for c in range(NCORES):
        b, hg = c // (NCORES // B), c % (NCORES // B)
        cols = slice(HC * hg, HC * (hg + 1))
        woT = Wo[:, cols].T.astype(h)  # [HC, D]
        in_maps.append(
            {
                "xqT": xT["q", b],
                "xkT": xT["k", b],
                "xvT": xT["v", b],
                "wqC": _chunk_w(Wq[cols, :].T.astype(h)),
                "wkC": _chunk_w(Wk[cols, :].T.astype(h)),
                "wvC": _chunk_w(Wv[cols, :].T.astype(h)),
                "woC": np.ascontiguousarray(
                    woT.reshape(2, P, D).transpose(1, 0, 2).reshape(P, 2 * D)
                ),
                "bqv": np.ascontiguousarray(bq[cols]),
                "bkv": np.ascontiguousarray(bk[cols]),
            }
        )
    return in_maps


def kernel(query, key, value, Wq, bq, Wk, bk, Wv, bv, Wo, bo, trace=False):
    nc = get_module()
    in_maps = shard_inputs(query, key, value, Wq, bq, Wk, bk, Wv, bv, Wo, bo)
    res = bass_utils.run_bass_kernel_spmd(
        nc, in_maps, core_ids=list(range(NCORES)), trace=trace
    )
    f = np.float32
    bias_term = np.asarray(bv, f) @ np.asarray(Wo, f).T + np.asarray(bo, f)
    output = np.empty((S, B, D), f)
    for b in range(B):
        acc = res.results[4 * b]["out"].astype(f)
        for c in range(4 * b + 1, 4 * b + 4):
            acc = acc + res.results[c]["out"]
        output[:, b, :] = acc + bias_term
    if trace:
        kernel.last_results = res
    return output


# revision 14
# speedup vs baseline: 1.1750x; 1.1750x over previous
"""MultiHeadAttention TRN2 Bass kernel (fp16 compute, pipelined schedule).

Problem: S=2048, B=2, H=16, d_k=64, D=1024, fp32 in/out.
  q = query @ Wq.T + bq ; k = key @ Wk.T + bk ; v = value @ Wv.T + bv
  score = einsum('qbhd,kbhd->qkbh', q, k) / 8 ; attn = softmax(score, axis=k)
  out = einsum('qkbh,kbhd->qbhd', attn, v) -> reshape -> @ Wo.T + bo

Sharding (8 cores): core c handles batch b = c//4 and heads [4*(c%4), 4*(c%4)+4).
Each core computes its partial output projection (tensor-parallel along the
head dim); the host sums the 4 partials per batch and adds the bias terms
(bv @ Wo.T + bo, the linear-foldable bias contributions).

All matmuls run fp16 (1 cycle/row on the PE; 2x the fp32r rate at 512-wide
moving dim). The ScalarE exp stream (128 x [128,1024], ~136us) is the hard
floor; the schedule aims to hide everything under it:
  - minimal prologue: only K(m0,tb0) + V(q0) + Q(m0,tb0) before attention
  - all other projections drain as fine-grained units between attention steps,
    ordered by the kb-iteration that first needs them (K m1 chunks are only
    needed by the second head-pair, 16 iterations later)
  - softmax normalization + output projection are deferred units too: the PV
    PSUM accumulator is freed immediately by a DVE copy, the rest (gpsimd
    partition-broadcast of the denominator row, DVE reciprocal + multiply,
    out-proj matmuls, DVE evac, DMA) drains under later exps
  - ScalarE does nothing but exp (plus one setup copy); copies/bias-adds run
    on DVE, the denominator broadcast on the otherwise idle GpSimd.
"""

import os

os.environ.setdefault("MYCRO_LOCAL_CACHE", "1")

from collections import deque

import numpy as np

import concourse.bass as bass
import concourse.tile as tile
from concourse import bacc, bass_utils, mybir


def _install_ntff_hook():
    """Provide antenv.axon_hooks when the image lacks it, so trace=True can
    capture NTFF profiles through the axon tunnel. Degrades silently."""
    import contextlib
    import ctypes
    import sys

    if "antenv.axon_hooks" in sys.modules:
        return
    so_path = "/opt/axon/libaxon_pjrt.so"
    if not os.path.exists(so_path):
        return
    try:
        lib = ctypes.CDLL(so_path)
        if not hasattr(lib, "axon_start_nrt_profile"):
            return
        lib.axon_start_nrt_profile.argtypes = [
            ctypes.POINTER(ctypes.c_int64),
            ctypes.c_size_t,
        ]
        lib.axon_start_nrt_profile.restype = ctypes.c_int64
        lib.axon_stop_nrt_profile.argtypes = [ctypes.c_char_p]
        lib.axon_stop_nrt_profile.restype = ctypes.c_int64

        @contextlib.contextmanager
        def _hook(output_dir, device_ids):
            import jax

            jax.devices()
            if device_ids:
                ids = (ctypes.c_int64 * len(device_ids))(*device_ids)
                rc = lib.axon_start_nrt_profile(ids, len(device_ids))
            else:
                rc = lib.axon_start_nrt_profile(None, 0)
            if rc != 0:
                raise RuntimeError(f"axon_start_nrt_profile rc={rc}")
            try:
                yield
            finally:
                n = lib.axon_stop_nrt_profile(str(output_dir).encode())
                print(f"ntff profile: {n} file(s) -> {output_dir}")

        import types

        mod = types.ModuleType("antenv.axon_hooks")
        mod.get_axon_ntff_profile_hook = lambda: _hook
        mod.set_axon_ntff_profile_hook = lambda h: None
        sys.modules["antenv.axon_hooks"] = mod
    except Exception:
        pass


_install_ntff_hook()

F32 = mybir.dt.float32
F16 = mybir.dt.float16
AF = mybir.ActivationFunctionType

S = 2048          # sequence length
B = 2             # batch
H = 16            # total heads
DK = 64           # head dim
D = 1024          # model dim
NCORES = 8
HL = H // (NCORES // B)   # heads per core = 4
HC = HL * DK              # head cols per core = 256
T = S                     # tokens per core (one batch element)
P = 128
QB = 512                  # q block (matmul free dim)
NKB = T // P              # 16 k blocks
NQB = T // QB             # 4 q blocks
NKC = D // P              # 8 contraction chunks for projections
VW = DK + 1               # 65: head value cols + ones column
EXP_BIAS = -2.0           # exp range shift; cancels in softmax normalization


def build_module():
    nc = bacc.Bacc("TRN2", target_bir_lowering=False, debug=False)

    xqT = nc.dram_tensor("xqT", [D, T], F16, kind="ExternalInput").ap()
    xkT = nc.dram_tensor("xkT", [D, T], F16, kind="ExternalInput").ap()
    xvT = nc.dram_tensor("xvT", [D, T], F16, kind="ExternalInput").ap()
    # weights pre-chunked on host to [128, kc/c, cols] partition-major layout
    wqC = nc.dram_tensor("wqC", [P, NKC * HC], F16, kind="ExternalInput").ap()
    wkC = nc.dram_tensor("wkC", [P, NKC * HC], F16, kind="ExternalInput").ap()
    wvC = nc.dram_tensor("wvC", [P, NKC * HC], F16, kind="ExternalInput").ap()
    woC = nc.dram_tensor("woC", [P, 2 * D], F16, kind="ExternalInput").ap()
    bqv = nc.dram_tensor("bqv", [HC], F32, kind="ExternalInput").ap()
    bkv = nc.dram_tensor("bkv", [HC], F32, kind="ExternalInput").ap()
    out = nc.dram_tensor("out", [T, D], F32, kind="ExternalOutput").ap()

    with tile.TileContext(nc) as tc:
        kernel_body(tc, xqT, xkT, xvT, wqC, wkC, wvC, woC, bqv, bkv, out)

    nc.compile()
    return nc


def kernel_body(tc, xqT, xkT, xvT, wqC, wkC, wvC, woC, bqv, bkv, out):
    nc = tc.nc

    with (
        tc.tile_pool(name="consts", bufs=1) as consts,
        tc.tile_pool(name="xk_p", bufs=1) as xk_p,
        tc.tile_pool(name="xq_p", bufs=1) as xq_p,
        tc.tile_pool(name="xv_p", bufs=1) as xv_p,
        tc.tile_pool(name="persist", bufs=1) as persist,
        tc.tile_pool(name="attn", bufs=6) as attn_pool,
        tc.tile_pool(name="small", bufs=4) as small,
        tc.tile_pool(name="outs", bufs=3) as outs,
        tc.tile_pool(name="ps_mm", bufs=2, space="PSUM") as ps_mm,
        tc.tile_pool(name="ps_sc", bufs=2, space="PSUM") as ps_sc,
        tc.tile_pool(name="ps_pv", bufs=2, space="PSUM") as ps_pv,
    ):
        # ---------------- constants (DMA-ordered: K weights first) ----------
        wk_s = consts.tile([P, NKC, HC], F16)
        nc.sync.dma_start(wk_s, wkC.rearrange("p (kc m) -> p kc m", kc=NKC))
        bk_s = consts.tile([P, HC // P], F32)
        nc.sync.dma_start(bk_s, bkv.rearrange("(m p) -> p m", p=P))

        ones16 = consts.tile([P, DK], F16)
        nc.vector.memset(ones16, 1.0)
        expb = consts.tile([P, 1], F32)
        nc.vector.memset(expb, EXP_BIAS)

        # ---------------- persistent activations ----------------
        QT = [persist.tile([P, T], F16, name=f"QT{m}") for m in range(2)]
        KT = [persist.tile([P, T], F16, name=f"KT{m}") for m in range(2)]
        V = persist.tile([P, NKB, HL * VW], F16, name="V")
        AC = [persist.tile([P, T], F16, name=f"AC{c}") for c in range(2)]

        # ones columns of V (denominator trick); one cheap ScalarE copy
        nc.scalar.activation(
            V.rearrange("p t (h c) -> p t h c", c=VW)[:, :, :, DK],
            ones16[:, : NKB * HL].rearrange("p (t h) -> p t h", h=HL),
            AF.Copy,
        )

        # x quarter tiles [128, 512].  xk/xq quarters stay resident (both m
        # chunks need them at different times); xv quarters rotate.
        xkt = {}  # (kc, tb) -> tile
        xqt = {}
        _xid = [0]

        def x_load(pool, store, xT, kc, tb, tag):
            _xid[0] += 1
            t = pool.tile([P, QB], F16, tag=f"{tag}{kc}_{tb}" if store is not None else "xv",
                          name=f"{tag}_{_xid[0]}")
            nc.sync.dma_start(t, xT[kc * P : (kc + 1) * P, tb * QB : (tb + 1) * QB])
            if store is not None:
                store[kc, tb] = t
            return t

        # ---------------- projection unit builders ----------------
        def qk_loads(store, pool, xT, tb, tag):
            return [
                (lambda kc=kc: x_load(pool, store, xT, kc, tb, tag))
                for kc in range(NKC)
            ]

        def qk_job(store, w_s, b_s, dst, tag, tb, m):
            # one (tb, m) projection job: psum alloc + 8 matmuls + bias evac
            stj = {}
            units = []

            def mk_start(stj=stj):
                stj["ps"] = ps_mm.tile([P, QB], F32, tag="mm", name=f"pj_{tag}{m}{tb}")

            units.append(mk_start)
            for kc in range(NKC):

                def mk_mm(kc=kc, stj=stj):
                    nc.tensor.matmul(
                        stj["ps"],
                        lhsT=w_s[:, kc, m * P : (m + 1) * P],
                        rhs=store[kc, tb],
                        start=(kc == 0),
                        stop=(kc == NKC - 1),
                    )

                units.append(mk_mm)

            def mk_evac(stj=stj):
                nc.vector.tensor_scalar_add(
                    dst[m][:, tb * QB : (tb + 1) * QB], stj["ps"], b_s[:, m : m + 1]
                )

            units.append(mk_evac)
            return units

        def v_units(tb):
            # V projection for quarter tb: 4 token blocks, each split into
            # two units (psum+4mm / 4mm+evac).  x tiles are pre-loaded.
            units = []
            st = {kc: None for kc in range(NKC)}
            for i in range(QB // P):
                t128 = tb * (QB // P) + i
                stj = {}

                def mk_a(i=i, stj=stj, tb=tb, t128=t128):
                    stj["ps"] = ps_mm.tile([P, HC], F32, tag="mm", name=f"pv_{t128}")
                    for kc in range(4):
                        nc.tensor.matmul(
                            stj["ps"],
                            lhsT=xvt[kc, tb][:, i * P : (i + 1) * P],
                            rhs=wv_s[:, kc, :],
                            start=(kc == 0),
                            stop=False,
                        )

                def mk_b(i=i, stj=stj, tb=tb, t128=t128):
                    for kc in range(4, NKC):
                        nc.tensor.matmul(
                            stj["ps"],
                            lhsT=xvt[kc, tb][:, i * P : (i + 1) * P],
                            rhs=wv_s[:, kc, :],
                            start=False,
                            stop=(kc == NKC - 1),
                        )
                    nc.vector.tensor_copy(
                        V[:, t128].rearrange("p (h c) -> p h c", c=VW)[:, :, :DK],
                        stj["ps"].rearrange("p (h c) -> p h c", c=DK),
                    )

                units.append(mk_a)
                units.append(mk_b)
            return units

        # ---------------- stage A ----------------
        # Issue ALL x-tile DMAs up front (the DMA engines run far ahead of
        # compute; this keeps drain-unit matmuls from blocking the in-order
        # PE queue on a just-issued transfer), then the minimal projections
        # attention iteration 0 needs.
        xvt = {}
        for u in qk_loads(xkt, xk_p, xkT, 0, "xk"):
            u()
        wv_s = consts.tile([P, NKC, HC], F16)
        nc.sync.dma_start(wv_s, wvC.rearrange("p (kc m) -> p kc m", kc=NKC))
        for u in qk_loads(xvt, xv_p, xvT, 0, "xv"):
            u()
        wq_s = consts.tile([P, NKC, HC], F16)
        nc.sync.dma_start(wq_s, wqC.rearrange("p (kc m) -> p kc m", kc=NKC))
        bq_s = consts.tile([P, HC // P], F32)
        nc.sync.dma_start(bq_s, bqv.rearrange("(m p) -> p m", p=P))
        for u in qk_loads(xqt, xq_p, xqT, 0, "xq"):
            u()
        for tb in (1, 2, 3):
            for u in qk_loads(xkt, xk_p, xkT, tb, "xk"):
                u()
            for u in qk_loads(xvt, xv_p, xvT, tb, "xv"):
                u()
            for u in qk_loads(xqt, xq_p, xqT, tb, "xq"):
                u()
        wo_s = consts.tile([P, HC // P, D], F16)
        nc.sync.dma_start(wo_s, woC.rearrange("p (c n) -> p c n", c=2))

        for u in qk_job(xkt, wk_s, bk_s, KT, "xk", 0, 0):
            u()
        for u in v_units(0):
            u()
        for u in qk_job(xqt, wq_s, bq_s, QT, "xq", 0, 0):
            u()

        # ---------------- zip queue: remaining projections ------------------
        zq = deque()
        for tb in (1, 2, 3):
            zq.extend(qk_job(xkt, wk_s, bk_s, KT, "xk", tb, 0))
            zq.extend(v_units(tb))
        zq.extend(qk_job(xqt, wq_s, bq_s, QT, "xq", 0, 1))
        for tb in (0, 1, 2, 3):
            zq.extend(qk_job(xkt, wk_s, bk_s, KT, "xk", tb, 1))
        for tb in (1, 2, 3):
            zq.extend(qk_job(xqt, wq_s, bq_s, QT, "xq", tb, 0))
            zq.extend(qk_job(xqt, wq_s, bq_s, QT, "xq", tb, 1))

        urgent = deque()  # deferred normalize / out-proj units (FIFO)

        def drain(n):
            for _ in range(n):
                if urgent:
                    urgent.popleft()()
                elif zq:
                    zq.popleft()()

        # ---------------- deferred normalize / output projection ------------
        def norm_units(qb, hp, pv0, pv1):
            # The pv-PSUM evacuation copies are emitted SYNCHRONOUSLY here
            # (the next head-pair's pv tiles re-bind these PSUM buffers at
            # emission time, so a deferred reader would race).  Only the
            # SBUF-side normalization work is deferred.
            m = hp
            st = {}
            for h, pv in ((2 * hp, pv0), (2 * hp + 1, pv1)):
                pvs = small.tile([VW, QB], F32, tag="pvs", name=f"pvs_{qb}_{h}")
                nc.vector.tensor_copy(pvs, pv)
                st[h] = pvs
                # denominator row re-homed to partition 0 (fp16) for the
                # PE outer-product broadcast
                dn = small.tile([1, QB], F32, tag="dn", name=f"dn_{qb}_{h}")
                nc.vector.tensor_copy(dn, pv[DK : DK + 1, :])
                st[h, "d"] = dn

            def bc_rcp(h, st=st):
                db = small.tile([DK, QB], F32, tag="db", name=f"db_{qb}_{h}")
                nc.gpsimd.partition_broadcast(db, st[h, "d"])
                rb = small.tile([DK, QB], F32, tag="rcp", name=f"rb_{qb}_{h}")
                nc.vector.reciprocal_approx_fast(rb, db)
                st[h, "r"] = rb

            def mul(h, st=st):
                off = 64 * (h % 2)
                nc.vector.tensor_mul(
                    AC[m][off : off + DK, qb * QB : (qb + 1) * QB],
                    st[h][:DK, :],
                    st[h, "r"],
                )

            h0, h1 = 2 * hp, 2 * hp + 1
            return [
                lambda: bc_rcp(h0),
                lambda: mul(h0),
                lambda: bc_rcp(h1),
                lambda: mul(h1),
            ]

        def oproj_units(qb):
            units = []
            for i in range(QB // P):
                t128 = qb * (QB // P) + i
                stj = {}

                def mk_mm(t128=t128, stj=stj):
                    stj["ps"] = [
                        ps_mm.tile([P, 512], F32, tag="mm", name=f"po_{t128}{n}")
                        for n in range(2)
                    ]
                    for c in range(2):
                        for n in range(2):
                            nc.tensor.matmul(
                                stj["ps"][n],
                                lhsT=AC[c][:, t128 * P : (t128 + 1) * P],
                                rhs=wo_s[:, c, n * 512 : (n + 1) * 512],
                                start=(c == 0),
                                stop=(c == 1),
                            )

                def mk_out(t128=t128, stj=stj):
                    ob = outs.tile([P, D], F32, tag="ob", name=f"ob_{t128}")
                    for n in range(2):
                        nc.vector.tensor_copy(ob[:, n * 512 : (n + 1) * 512],
                                              stj["ps"][n])
                    nc.sync.dma_start(out[t128 * P : (t128 + 1) * P, :], ob)

                units.append(mk_mm)
                units.append(mk_out)
            return units

        # ---------------- attention ----------------
        it = [0]

        def drain_n():
            i = it[0]
            it[0] += 1
            return 6 if i < 16 else (4 if i < 32 else 2)

        for qb in range(NQB):
            for hp in range(2):
                m = hp  # heads (2*hp, 2*hp+1) live in QT/KT chunk m
                h0, h1 = 2 * hp, 2 * hp + 1
                pv0 = ps_pv.tile([VW, QB], F32, tag="pv", name=f"pv_{qb}_{h0}")
                pv1 = ps_pv.tile([VW, QB], F32, tag="pv", name=f"pv_{qb}_{h1}")

                def emit_pv(kb, at, pv0=pv0, pv1=pv1, h0=h0, h1=h1):
                    nc.tensor.matmul(
                        pv0,
                        lhsT=V[:, kb, VW * h0 : VW * (h0 + 1)],
                        rhs=at[:, :QB],
                        start=(kb == 0),
                        stop=(kb == NKB - 1),
                    )
                    nc.tensor.matmul(
                        pv1,
                        lhsT=V[:, kb, VW * h1 : VW * (h1 + 1)],
                        rhs=at[:, QB:],
                        start=(kb == 0),
                        stop=(kb == NKB - 1),
                    )

                # Software-pipelined: PV for block kb issues after the score
                # pair for kb+1, giving the exp a full score-pair of slack.
                prev = None
                for kb in range(NKB):
                    sc = ps_sc.tile(
                        [P, 2 * QB], F32, tag="sc", name=f"sc_{qb}_{hp}_{kb}"
                    )
                    nc.tensor.matmul(
                        sc[:, :QB],
                        lhsT=KT[m][0:DK, kb * P : (kb + 1) * P],
                        rhs=QT[m][0:DK, qb * QB : (qb + 1) * QB],
                        start=True,
                        stop=True,
                    )
                    nc.tensor.matmul(
                        sc[:, QB:],
                        lhsT=KT[m][DK:P, kb * P : (kb + 1) * P],
                        rhs=QT[m][DK:P, qb * QB : (qb + 1) * QB],
                        start=True,
                        stop=True,
                    )
                    at = attn_pool.tile(
                        [P, 2 * QB], F16, tag="at", name=f"at_{qb}_{hp}_{kb}"
                    )
                    nc.scalar.activation(at, sc, AF.Exp, scale=0.125, bias=expb)
                    if prev is not None:
                        emit_pv(*prev)
                    prev = (kb, at)
                    drain(drain_n())
                emit_pv(*prev)

                urgent.extend(norm_units(qb, hp, pv0, pv1))
            urgent.extend(oproj_units(qb))

        while urgent or zq:
            drain(8)


_module_cache = None


def get_module():
    global _module_cache
    if _module_cache is None:
        _module_cache = build_module()
    return _module_cache


def _chunk_w(wT):
    # [D, HC] -> [128, NKC*HC]: partition-major kc blocks, contiguous rows
    return np.ascontiguousarray(
        wT.reshape(NKC, P, HC).transpose(1, 0, 2).reshape(P, NKC * HC)
    )


def shard_inputs(query, key, value, Wq, bq, Wk, bk, Wv, bv, Wo, bo):
    """Build the 8 per-core input maps (host-side layout/dtype transforms)."""
    f = np.float32
    h = np.float16
    xT = {}
    for b in range(B):
        xT["q", b] = np.ascontiguousarray(np.asarray(query)[:, b, :].T.astype(h))
        xT["k", b] = np.ascontiguousarray(np.asarray(key)[:, b, :].T.astype(h))
        xT["v", b] = np.ascontiguousarray(np.asarray(value)[:, b, :].T.astype(h))
    Wq, Wk, Wv, Wo = (np.asarray(w, f) for w in (Wq, Wk, Wv, Wo))
    bq, bk = np.asarray(bq, f), np.asarray(bk, f)
    in_maps = []
    for c in range(NCORES):
        b, hg = c // (NCORES // B), c % (NCORES // B)
        cols = slice(HC * hg, HC * (hg + 1))
        woT = Wo[:, cols].T.astype(h)  # [HC, D]
        in_maps.append(
            {
                "xqT": xT["q", b],
                "xkT": xT["k", b],
                "xvT": xT["v", b],
                "wqC": _chunk_w(Wq[cols, :].T.astype(h)),
                "wkC": _chunk_w(Wk[cols, :].T.astype(h)),
                "wvC": _chunk_w(Wv[cols, :].T.astype(h)),
                "woC": np.ascontiguousarray(
                    woT.reshape(2, P, D).transpose(1, 0, 2).reshape(P, 2 * D)
                ),
                "bqv": np.ascontiguousarray(bq[cols]),
                "bkv": np.ascontiguousarray(bk[cols]),
            }
        )
    return in_maps


def kernel(query, key, value, Wq, bq, Wk, bk, Wv, bv, Wo, bo, trace=False):
    nc = get_module()
    in_maps = shard_inputs(query, key, value, Wq, bq, Wk, bk, Wv, bv, Wo, bo)
    res = bass_utils.run_bass_kernel_spmd(
        nc, in_maps, core_ids=list(range(NCORES)), trace=trace
    )
    f = np.float32
    bias_term = np.asarray(bv, f) @ np.asarray(Wo, f).T + np.asarray(bo, f)
    output = np.empty((S, B, D), f)
    for b in range(B):
        acc = res.results[4 * b]["out"].astype(f)
        for c in range(4 * b + 1, 4 * b + 4):
            acc = acc + res.results[c]["out"]
        output[:, b, :] = acc + bias_term
    if trace:
        kernel.last_results = res
    return output


# revision 16
# speedup vs baseline: 1.2978x; 1.1045x over previous
"""MultiHeadAttention TRN2 Bass kernel (fp16 compute, pipelined schedule).

Problem: S=2048, B=2, H=16, d_k=64, D=1024, fp32 in/out.
  q = query @ Wq.T + bq ; k = key @ Wk.T + bk ; v = value @ Wv.T + bv
  score = einsum('qbhd,kbhd->qkbh', q, k) / 8 ; attn = softmax(score, axis=k)
  out = einsum('qkbh,kbhd->qbhd', attn, v) -> reshape -> @ Wo.T + bo

Sharding (8 cores): core c handles batch b = c//4 and heads [4*(c%4), 4*(c%4)+4).
Each core computes its partial output projection (tensor-parallel along the
head dim); the host sums the 4 partials per batch and adds the bias terms
(bv @ Wo.T + bo, the linear-foldable bias contributions).

All matmuls run fp16 (1 cycle/row on the PE; 2x the fp32r rate at 512-wide
moving dim). The ScalarE exp stream (128 x [128,1024], ~136us) is the hard
floor; the schedule aims to hide everything under it:
  - minimal prologue: only K(m0,tb0) + V(q0) + Q(m0,tb0) before attention
  - all other projections drain as fine-grained units between attention steps,
    ordered by the kb-iteration that first needs them (K m1 chunks are only
    needed by the second head-pair, 16 iterations later)
  - softmax normalization + output projection are deferred units too: the PV
    PSUM accumulator is freed immediately by a DVE copy, the rest (gpsimd
    partition-broadcast of the denominator row, DVE reciprocal + multiply,
    out-proj matmuls, DVE evac, DMA) drains under later exps
  - ScalarE does nothing but exp (plus one setup copy); copies/bias-adds run
    on DVE, the denominator broadcast on the otherwise idle GpSimd.
"""

import os

os.environ.setdefault("MYCRO_LOCAL_CACHE", "1")

from collections import deque

import numpy as np

import concourse.bass as bass
import concourse.tile as tile
from concourse import bacc, bass_utils, mybir


def _install_ntff_hook():
    """Provide antenv.axon_hooks when the image lacks it, so trace=True can
    capture NTFF profiles through the axon tunnel. Degrades silently."""
    import contextlib
    import ctypes
    import sys

    if "antenv.axon_hooks" in sys.modules:
        return
    so_path = "/opt/axon/libaxon_pjrt.so"
    if not os.path.exists(so_path):
        return
    try:
        lib = ctypes.CDLL(so_path)
        if not hasattr(lib, "axon_start_nrt_profile"):
            return
        lib.axon_start_nrt_profile.argtypes = [
            ctypes.POINTER(ctypes.c_int64),
            ctypes.c_size_t,
        ]
        lib.axon_start_nrt_profile.restype = ctypes.c_int64
        lib.axon_stop_nrt_profile.argtypes = [ctypes.c_char_p]
        lib.axon_stop_nrt_profile.restype = ctypes.c_int64

        @contextlib.contextmanager
        def _hook(output_dir, device_ids):
            import jax

            jax.devices()
            if device_ids:
                ids = (ctypes.c_int64 * len(device_ids))(*device_ids)
                rc = lib.axon_start_nrt_profile(ids, len(device_ids))
            else:
                rc = lib.axon_start_nrt_profile(None, 0)
            if rc != 0:
                raise RuntimeError(f"axon_start_nrt_profile rc={rc}")
            try:
                yield
            finally:
                n = lib.axon_stop_nrt_profile(str(output_dir).encode())
                print(f"ntff profile: {n} file(s) -> {output_dir}")

        import types

        mod = types.ModuleType("antenv.axon_hooks")
        mod.get_axon_ntff_profile_hook = lambda: _hook
        mod.set_axon_ntff_profile_hook = lambda h: None
        sys.modules["antenv.axon_hooks"] = mod
    except Exception:
        pass


_install_ntff_hook()

F32 = mybir.dt.float32
F16 = mybir.dt.float16
AF = mybir.ActivationFunctionType

S = 2048          # sequence length
B = 2             # batch
H = 16            # total heads
DK = 64           # head dim
D = 1024          # model dim
NCORES = 8
HL = H // (NCORES // B)   # heads per core = 4
HC = HL * DK              # head cols per core = 256
T = S                     # tokens per core (one batch element)
P = 128
QB = 512                  # q block (matmul free dim)
NKB = T // P              # 16 k blocks
NQB = T // QB             # 4 q blocks
NKC = D // P              # 8 contraction chunks for projections
VW = DK + 1               # 65: head value cols + ones column
EXP_BIAS = -2.0           # exp range shift; cancels in softmax normalization


def build_module():
    nc = bacc.Bacc("TRN2", target_bir_lowering=False, debug=False)

    xqT = nc.dram_tensor("xqT", [D, T], F16, kind="ExternalInput").ap()
    xkT = nc.dram_tensor("xkT", [D, T], F16, kind="ExternalInput").ap()
    xvT = nc.dram_tensor("xvT", [D, T], F16, kind="ExternalInput").ap()
    # weights pre-chunked on host to [128, kc/c, cols] partition-major layout
    wqC = nc.dram_tensor("wqC", [P, NKC * HC], F16, kind="ExternalInput").ap()
    wkC = nc.dram_tensor("wkC", [P, NKC * HC], F16, kind="ExternalInput").ap()
    wvC = nc.dram_tensor("wvC", [P, NKC * HC], F16, kind="ExternalInput").ap()
    woC = nc.dram_tensor("woC", [P, 2 * D], F16, kind="ExternalInput").ap()
    bqv = nc.dram_tensor("bqv", [HC], F32, kind="ExternalInput").ap()
    bkv = nc.dram_tensor("bkv", [HC], F32, kind="ExternalInput").ap()
    out = nc.dram_tensor("out", [T, D], F32, kind="ExternalOutput").ap()

    with tile.TileContext(nc) as tc:
        kernel_body(tc, xqT, xkT, xvT, wqC, wkC, wvC, woC, bqv, bkv, out)

    nc.compile()
    return nc


def kernel_body(tc, xqT, xkT, xvT, wqC, wkC, wvC, woC, bqv, bkv, out):
    nc = tc.nc

    with (
        tc.tile_pool(name="consts", bufs=1) as consts,
        tc.tile_pool(name="xk_p", bufs=1) as xk_p,
        tc.tile_pool(name="xq_p", bufs=1) as xq_p,
        tc.tile_pool(name="xv_p", bufs=1) as xv_p,
        tc.tile_pool(name="persist", bufs=1) as persist,
        tc.tile_pool(name="attn", bufs=6) as attn_pool,
        tc.tile_pool(name="small", bufs=4) as small,
        tc.tile_pool(name="outs", bufs=3) as outs,
        tc.tile_pool(name="ps_mm", bufs=2, space="PSUM") as ps_mm,
        tc.tile_pool(name="ps_sc", bufs=2, space="PSUM") as ps_sc,
        tc.tile_pool(name="ps_pv", bufs=2, space="PSUM") as ps_pv,
    ):
        # ---------------- constants (DMA-ordered: K weights first) ----------
        wk_s = consts.tile([P, NKC, HC], F16)
        nc.sync.dma_start(wk_s, wkC.rearrange("p (kc m) -> p kc m", kc=NKC))
        bk_s = consts.tile([P, HC // P], F32)
        nc.sync.dma_start(bk_s, bkv.rearrange("(m p) -> p m", p=P))

        ones16 = consts.tile([P, DK], F16)
        nc.vector.memset(ones16, 1.0)
        expb = consts.tile([P, 1], F32)
        nc.vector.memset(expb, EXP_BIAS)

        # ---------------- persistent activations ----------------
        QT = [persist.tile([P, T], F16, name=f"QT{m}") for m in range(2)]
        KT = [persist.tile([P, T], F16, name=f"KT{m}") for m in range(2)]
        V = persist.tile([P, NKB, HL * VW], F16, name="V")
        AC = [persist.tile([P, T], F16, name=f"AC{c}") for c in range(2)]

        # ones columns of V (denominator trick); one cheap ScalarE copy
        nc.scalar.activation(
            V.rearrange("p t (h c) -> p t h c", c=VW)[:, :, :, DK],
            ones16[:, : NKB * HL].rearrange("p (t h) -> p t h", h=HL),
            AF.Copy,
        )

        # x quarter tiles [128, NKC, 512], one batched DMA each (few sync-
        # engine ops; 1KB descriptors).  All stay resident.
        xkt = {}  # tb -> tile
        xqt = {}
        xvt = {}

        def x_load(pool, store, xT, tb, tag):
            t = pool.tile([P, NKC, QB], F16, tag=f"{tag}_{tb}", name=f"{tag}_{tb}")
            nc.sync.dma_start(
                t,
                xT.rearrange("(kc p) t -> p kc t", p=P)[
                    :, :, tb * QB : (tb + 1) * QB
                ],
            )
            store[tb] = t

        # ---------------- projection unit builders ----------------
        def qk_job(store, w_s, b_s, dst, tag, tb, m):
            # one (tb, m) projection job: psum alloc + 8 matmuls + bias evac
            stj = {}
            units = []

            def mk_start(stj=stj):
                stj["ps"] = ps_mm.tile([P, QB], F32, tag="mm", name=f"pj_{tag}{m}{tb}")

            units.append(mk_start)
            for kc in range(NKC):

                def mk_mm(kc=kc, stj=stj):
                    nc.tensor.matmul(
                        stj["ps"],
                        lhsT=w_s[:, kc, m * P : (m + 1) * P],
                        rhs=store[tb][:, kc, :],
                        start=(kc == 0),
                        stop=(kc == NKC - 1),
                    )

                units.append(mk_mm)

            def mk_evac(stj=stj):
                nc.vector.tensor_scalar_add(
                    dst[m][:, tb * QB : (tb + 1) * QB], stj["ps"], b_s[:, m : m + 1]
                )

            units.append(mk_evac)
            return units

        def v_units(tb):
            # V projection for quarter tb: 4 token blocks, each split into
            # two units (psum+4mm / 4mm+evac).  x tiles are pre-loaded.
            units = []
            for i in range(QB // P):
                t128 = tb * (QB // P) + i
                stj = {}

                def mk_a(i=i, stj=stj, tb=tb, t128=t128):
                    stj["ps"] = ps_mm.tile([P, HC], F32, tag="mm", name=f"pv_{t128}")
                    for kc in range(4):
                        nc.tensor.matmul(
                            stj["ps"],
                            lhsT=xvt[tb][:, kc, i * P : (i + 1) * P],
                            rhs=wv_s[:, kc, :],
                            start=(kc == 0),
                            stop=False,
                        )

                def mk_b(i=i, stj=stj, tb=tb, t128=t128):
                    for kc in range(4, NKC):
                        nc.tensor.matmul(
                            stj["ps"],
                            lhsT=xvt[tb][:, kc, i * P : (i + 1) * P],
                            rhs=wv_s[:, kc, :],
                            start=False,
                            stop=(kc == NKC - 1),
                        )
                    nc.vector.tensor_copy(
                        V[:, t128].rearrange("p (h c) -> p h c", c=VW)[:, :, :DK],
                        stj["ps"].rearrange("p (h c) -> p h c", c=DK),
                    )

                units.append(mk_a)
                units.append(mk_b)
            return units

        # ---------------- stage A ----------------
        # Issue all x-tile DMAs up front, ordered by when compute needs them
        # (the DMA engines run far ahead; drain-unit matmuls then never block
        # the in-order PE queue on a just-issued transfer).
        x_load(xk_p, xkt, xkT, 0, "xk")
        wv_s = consts.tile([P, NKC, HC], F16)
        nc.sync.dma_start(wv_s, wvC.rearrange("p (kc m) -> p kc m", kc=NKC))
        x_load(xv_p, xvt, xvT, 0, "xv")
        wq_s = consts.tile([P, NKC, HC], F16)
        nc.sync.dma_start(wq_s, wqC.rearrange("p (kc m) -> p kc m", kc=NKC))
        bq_s = consts.tile([P, HC // P], F32)
        nc.sync.dma_start(bq_s, bqv.rearrange("(m p) -> p m", p=P))
        x_load(xq_p, xqt, xqT, 0, "xq")
        for tb in (1, 2, 3):
            x_load(xk_p, xkt, xkT, tb, "xk")
            x_load(xv_p, xvt, xvT, tb, "xv")
        for tb in (1, 2, 3):
            x_load(xq_p, xqt, xqT, tb, "xq")
        wo_s = consts.tile([P, HC // P, D], F16)
        nc.sync.dma_start(wo_s, woC.rearrange("p (c n) -> p c n", c=2))

        for u in qk_job(xkt, wk_s, bk_s, KT, "xk", 0, 0):
            u()
        for u in v_units(0):
            u()
        for u in qk_job(xqt, wq_s, bq_s, QT, "xq", 0, 0):
            u()

        # ---------------- zip queue: remaining projections ------------------
        zq = deque()
        for tb in (1, 2, 3):
            zq.extend(qk_job(xkt, wk_s, bk_s, KT, "xk", tb, 0))
            zq.extend(v_units(tb))
        zq.extend(qk_job(xqt, wq_s, bq_s, QT, "xq", 0, 1))
        for tb in (0, 1, 2, 3):
            zq.extend(qk_job(xkt, wk_s, bk_s, KT, "xk", tb, 1))
        for tb in (1, 2, 3):
            zq.extend(qk_job(xqt, wq_s, bq_s, QT, "xq", tb, 0))
            zq.extend(qk_job(xqt, wq_s, bq_s, QT, "xq", tb, 1))

        urgent = deque()  # deferred normalize / out-proj units (FIFO)

        def drain(n):
            for _ in range(n):
                if urgent:
                    urgent.popleft()()
                elif zq:
                    zq.popleft()()

        # ---------------- deferred normalize / output projection ------------
        def norm_finish(qb, hp, pv0, pv1, last):
            # The pv-PSUM evacuation copies are emitted SYNCHRONOUSLY here
            # (the next head-pair's pv tiles re-bind these PSUM buffers at
            # emission time, so a deferred reader would race).  Only the
            # SBUF-side normalization work is deferred.  On the final group
            # the copies are split across ScalarE (idle, exps done) and DVE.
            m = hp
            st = {}
            for j, (h, pv) in enumerate(((2 * hp, pv0), (2 * hp + 1, pv1))):
                pvs = small.tile([VW, QB], F32, tag="pvs", name=f"pvs_{qb}_{h}")
                dn = small.tile([1, QB], F32, tag="dn", name=f"dn_{qb}_{h}")
                if last and j == 1:
                    nc.scalar.copy(pvs, pv)
                    nc.scalar.copy(dn, pv[DK : DK + 1, :])
                else:
                    nc.vector.tensor_copy(pvs, pv)
                    nc.vector.tensor_copy(dn, pv[DK : DK + 1, :])
                st[h] = pvs
                st[h, "d"] = dn

            def bc_rcp(h, st=st):
                db = small.tile([DK, QB], F32, tag="db", name=f"db_{qb}_{h}")
                nc.gpsimd.partition_broadcast(db, st[h, "d"])
                rb = small.tile([DK, QB], F32, tag="rcp", name=f"rb_{qb}_{h}")
                nc.vector.reciprocal_approx_fast(rb, db)
                st[h, "r"] = rb

            def mul(h, st=st):
                off = 64 * (h % 2)
                nc.vector.tensor_mul(
                    AC[m][off : off + DK, qb * QB : (qb + 1) * QB],
                    st[h][:DK, :],
                    st[h, "r"],
                )

            h0, h1 = 2 * hp, 2 * hp + 1
            return [
                lambda: bc_rcp(h0),
                lambda: mul(h0),
                lambda: bc_rcp(h1),
                lambda: mul(h1),
            ]

        def oproj_units(qb, last):
            units = []
            for i in range(QB // P):
                t128 = qb * (QB // P) + i
                stj = {}

                def mk_mm(n, t128=t128, stj=stj):
                    ps = ps_mm.tile([P, 512], F32, tag="mm", name=f"po_{t128}{n}")
                    stj[n] = ps
                    for c in range(2):
                        nc.tensor.matmul(
                            ps,
                            lhsT=AC[c][:, t128 * P : (t128 + 1) * P],
                            rhs=wo_s[:, c, n * 512 : (n + 1) * 512],
                            start=(c == 0),
                            stop=(c == 1),
                        )

                def mk_ev(n, t128=t128, stj=stj, i=i):
                    if n == 0:
                        stj["ob"] = outs.tile([P, D], F32, tag="ob",
                                              name=f"ob_{t128}")
                    eng = nc.scalar if (last and (i + n) % 2) else nc.vector
                    if eng is nc.scalar:
                        nc.scalar.copy(stj["ob"][:, n * 512 : (n + 1) * 512],
                                       stj[n])
                    else:
                        nc.vector.tensor_copy(
                            stj["ob"][:, n * 512 : (n + 1) * 512], stj[n]
                        )
                    if n == 1:
                        nc.sync.dma_start(
                            out[t128 * P : (t128 + 1) * P, :], stj["ob"]
                        )

                units.append(lambda stj=stj, t128=t128: mk_mm(0, t128, stj))
                units.append(lambda stj=stj, t128=t128: mk_ev(0, t128, stj))
                units.append(lambda stj=stj, t128=t128: mk_mm(1, t128, stj))
                units.append(lambda stj=stj, t128=t128: mk_ev(1, t128, stj))
            return units

        # ---------------- attention ----------------
        # One flat software-pipelined stream over all (qb, hp) groups: the
        # first score pair of group g+1 is emitted BEFORE the last PV of
        # group g, so the exp stream never waits on a PV->sc turnaround.
        it = [0]

        def drain_n():
            i = it[0]
            it[0] += 1
            return 6 if i < 16 else (4 if i < 32 else 2)

        groups = [(qb, hp) for qb in range(NQB) for hp in range(2)]
        prev = None        # (emit_fn,) pending PV pair
        finish = None      # pending group-finish (pv evac + deferred pushes)
        for gi, (qb, hp) in enumerate(groups):
            m = hp  # heads (2*hp, 2*hp+1) live in QT/KT chunk m
            h0, h1 = 2 * hp, 2 * hp + 1
            box = {}

            def mk_emit(kb, at, box=box, h0=h0, h1=h1):
                def emit():
                    nc.tensor.matmul(
                        box["pv0"],
                        lhsT=V[:, kb, VW * h0 : VW * (h0 + 1)],
                        rhs=at[:, :QB],
                        start=(kb == 0),
                        stop=(kb == NKB - 1),
                    )
                    nc.tensor.matmul(
                        box["pv1"],
                        lhsT=V[:, kb, VW * h1 : VW * (h1 + 1)],
                        rhs=at[:, QB:],
                        start=(kb == 0),
                        stop=(kb == NKB - 1),
                    )

                return emit

            for kb in range(NKB):
                sc = ps_sc.tile(
                    [P, 2 * QB], F32, tag="sc", name=f"sc_{qb}_{hp}_{kb}"
                )
                nc.tensor.matmul(
                    sc[:, :QB],
                    lhsT=KT[m][0:DK, kb * P : (kb + 1) * P],
                    rhs=QT[m][0:DK, qb * QB : (qb + 1) * QB],
                    start=True,
                    stop=True,
                )
                nc.tensor.matmul(
                    sc[:, QB:],
                    lhsT=KT[m][DK:P, kb * P : (kb + 1) * P],
                    rhs=QT[m][DK:P, qb * QB : (qb + 1) * QB],
                    start=True,
                    stop=True,
                )
                at = attn_pool.tile(
                    [P, 2 * QB], F16, tag="at", name=f"at_{qb}_{hp}_{kb}"
                )
                nc.scalar.activation(at, sc, AF.Exp, scale=0.125, bias=expb)
                if prev is not None:
                    prev()
                    prev = None
                if finish is not None:
                    finish()
                    finish = None
                if "pv0" not in box:
                    box["pv0"] = ps_pv.tile(
                        [VW, QB], F32, tag="pv", name=f"pv_{qb}_{h0}"
                    )
                    box["pv1"] = ps_pv.tile(
                        [VW, QB], F32, tag="pv", name=f"pv_{qb}_{h1}"
                    )
                prev = mk_emit(kb, at)
                drain(drain_n())

            def mk_finish(qb=qb, hp=hp, box=box, gi=gi):
                def fin():
                    last = gi == len(groups) - 1
                    urgent.extend(
                        norm_finish(qb, hp, box["pv0"], box["pv1"], last)
                    )
                    if hp == 1:
                        urgent.extend(oproj_units(qb, last))

                return fin

            finish = mk_finish()

        prev()
        finish()
        while urgent or zq:
            drain(8)


_module_cache = None


def get_module():
    global _module_cache
    if _module_cache is None:
        _module_cache = build_module()
    return _module_cache


def _chunk_w(wT):
    # [D, HC] -> [128, NKC*HC]: partition-major kc blocks, contiguous rows
    return np.ascontiguousarray(
        wT.reshape(NKC, P, HC).transpose(1, 0, 2).reshape(P, NKC * HC)
    )


def shard_inputs(query, key, value, Wq, bq, Wk, bk, Wv, bv, Wo, bo):
    """Build the 8 per-core input maps (host-side layout/dtype transforms)."""
    f = np.float32
    h = np.float16
    xT = {}
    for b in range(B):
        xT["q", b] = np.ascontiguousarray(np.asarray(query)[:, b, :].T.astype(h))
        xT["k", b] = np.ascontiguousarray(np.asarray(key)[:, b, :].T.astype(h))
        xT["v", b] = np.ascontiguousarray(np.asarray(value)[:, b, :].T.astype(h))
    Wq, Wk, Wv, Wo = (np.asarray(w, f) for w in (Wq, Wk, Wv, Wo))
    bq, bk = np.asarray(bq, f), np.asarray(bk, f)
    in_maps = []
    for c in range(NCORES):
        b, hg = c // (NCORES // B), c % (NCORES // B)
        cols = slice(HC * hg, HC * (hg + 1))
        woT = Wo[:, cols].T.astype(h)  # [HC, D]
        in_maps.append(
            {
                "xqT": xT["q", b],
                "xkT": xT["k", b],
                "xvT": xT["v", b],
                "wqC": _chunk_w(Wq[cols, :].T.astype(h)),
                "wkC": _chunk_w(Wk[cols, :].T.astype(h)),
                "wvC": _chunk_w(Wv[cols, :].T.astype(h)),
                "woC": np.ascontiguousarray(
                    woT.reshape(2, P, D).transpose(1, 0, 2).reshape(P, 2 * D)
                ),
                "bqv": np.ascontiguousarray(bq[cols]),
                "bkv": np.ascontiguousarray(bk[cols]),
            }
        )
    return in_maps


def kernel(query, key, value, Wq, bq, Wk, bk, Wv, bv, Wo, bo, trace=False):
    nc = get_module()
    in_maps = shard_inputs(query, key, value, Wq, bq, Wk, bk, Wv, bv, Wo, bo)
    res = bass_utils.run_bass_kernel_spmd(
        nc, in_maps, core_ids=list(range(NCORES)), trace=trace
    )
    f = np.float32
    bias_term = np.asarray(bv, f) @ np.asarray(Wo, f).T + np.asarray(bo, f)
    output = np.empty((S, B, D), f)
    for b in range(B):
        acc = res.results[4 * b]["out"].astype(f)
        for c in range(4 * b + 1, 4 * b + 4):
            acc = acc + res.results[c]["out"]
        output[:, b, :] = acc + bias_term
    if trace:
        kernel.last_results = res
    return output


# revision 26
# speedup vs baseline: 1.3155x; 1.0136x over previous
"""MultiHeadAttention TRN2 Bass kernel (fp16 compute, pipelined schedule).

Problem: S=2048, B=2, H=16, d_k=64, D=1024, fp32 in/out.
  q = query @ Wq.T + bq ; k = key @ Wk.T + bk ; v = value @ Wv.T + bv
  score = einsum('qbhd,kbhd->qkbh', q, k) / 8 ; attn = softmax(score, axis=k)
  out = einsum('qkbh,kbhd->qbhd', attn, v) -> reshape -> @ Wo.T + bo

Sharding (8 cores): core c handles batch b = c//4 and heads [4*(c%4), 4*(c%4)+4).
Each core computes its partial output projection (tensor-parallel along the
head dim); the host sums the 4 partials per batch and adds the bias terms
(bv @ Wo.T + bo, the linear-foldable bias contributions).

All matmuls run fp16 (1 cycle/row on the PE; 2x the fp32r rate at 512-wide
moving dim). The ScalarE exp stream (128 x [128,1024], ~136us) is the hard
floor; the schedule aims to hide everything under it:
  - minimal prologue: only K(m0,tb0) + V(q0) + Q(m0,tb0) before attention
  - all other projections drain as fine-grained units between attention steps,
    ordered by the kb-iteration that first needs them (K m1 chunks are only
    needed by the second head-pair, 16 iterations later)
  - softmax normalization + output projection are deferred units too: the PV
    PSUM accumulator is freed immediately by a DVE copy, the rest (gpsimd
    partition-broadcast of the denominator row, DVE reciprocal + multiply,
    out-proj matmuls, DVE evac, DMA) drains under later exps
  - ScalarE does nothing but exp (plus one setup copy); copies/bias-adds run
    on DVE, the denominator broadcast on the otherwise idle GpSimd.
"""

import os

os.environ.setdefault("MYCRO_LOCAL_CACHE", "1")

from collections import deque

import numpy as np

import concourse.bass as bass
import concourse.tile as tile
from concourse import bacc, bass_utils, mybir


def _install_ntff_hook():
    """Provide antenv.axon_hooks when the image lacks it, so trace=True can
    capture NTFF profiles through the axon tunnel. Degrades silently."""
    import contextlib
    import ctypes
    import sys

    if "antenv.axon_hooks" in sys.modules:
        return
    so_path = "/opt/axon/libaxon_pjrt.so"
    if not os.path.exists(so_path):
        return
    try:
        lib = ctypes.CDLL(so_path)
        if not hasattr(lib, "axon_start_nrt_profile"):
            return
        lib.axon_start_nrt_profile.argtypes = [
            ctypes.POINTER(ctypes.c_int64),
            ctypes.c_size_t,
        ]
        lib.axon_start_nrt_profile.restype = ctypes.c_int64
        lib.axon_stop_nrt_profile.argtypes = [ctypes.c_char_p]
        lib.axon_stop_nrt_profile.restype = ctypes.c_int64

        @contextlib.contextmanager
        def _hook(output_dir, device_ids):
            import jax

            jax.devices()
            if device_ids:
                ids = (ctypes.c_int64 * len(device_ids))(*device_ids)
                rc = lib.axon_start_nrt_profile(ids, len(device_ids))
            else:
                rc = lib.axon_start_nrt_profile(None, 0)
            if rc != 0:
                raise RuntimeError(f"axon_start_nrt_profile rc={rc}")
            try:
                yield
            finally:
                n = lib.axon_stop_nrt_profile(str(output_dir).encode())
                print(f"ntff profile: {n} file(s) -> {output_dir}")

        import types

        mod = types.ModuleType("antenv.axon_hooks")
        mod.get_axon_ntff_profile_hook = lambda: _hook
        mod.set_axon_ntff_profile_hook = lambda h: None
        sys.modules["antenv.axon_hooks"] = mod
    except Exception:
        pass


_install_ntff_hook()

F32 = mybir.dt.float32
F16 = mybir.dt.float16
AF = mybir.ActivationFunctionType

S = 2048          # sequence length
B = 2             # batch
H = 16            # total heads
DK = 64           # head dim
D = 1024          # model dim
NCORES = 8
HL = H // (NCORES // B)   # heads per core = 4
HC = HL * DK              # head cols per core = 256
T = S                     # tokens per core (one batch element)
P = 128
QB = 512                  # q block (matmul free dim)
NKB = T // P              # 16 k blocks
NQB = T // QB             # 4 q blocks
NKC = D // P              # 8 contraction chunks for projections
VW = DK + 1               # 65: head value cols + ones column
EXP_BIAS = -2.0           # exp range shift; cancels in softmax normalization


def build_module():
    nc = bacc.Bacc("TRN2", target_bir_lowering=False, debug=False)

    xqT = nc.dram_tensor("xqT", [D, T], F16, kind="ExternalInput").ap()
    xkT = nc.dram_tensor("xkT", [D, T], F16, kind="ExternalInput").ap()
    xvT = nc.dram_tensor("xvT", [D, T], F16, kind="ExternalInput").ap()
    # weights pre-chunked on host to [128, kc/c, cols] partition-major layout
    wqC = nc.dram_tensor("wqC", [P, NKC * HC], F16, kind="ExternalInput").ap()
    wkC = nc.dram_tensor("wkC", [P, NKC * HC], F16, kind="ExternalInput").ap()
    wvC = nc.dram_tensor("wvC", [P, NKC * HC], F16, kind="ExternalInput").ap()
    woC = nc.dram_tensor("woC", [P, 2 * D], F16, kind="ExternalInput").ap()
    bqv = nc.dram_tensor("bqv", [HC], F32, kind="ExternalInput").ap()
    bkv = nc.dram_tensor("bkv", [HC], F32, kind="ExternalInput").ap()
    out = nc.dram_tensor("out", [T, D], F32, kind="ExternalOutput").ap()

    with tile.TileContext(nc) as tc:
        kernel_body(tc, xqT, xkT, xvT, wqC, wkC, wvC, woC, bqv, bkv, out)

    nc.compile()
    return nc


def kernel_body(tc, xqT, xkT, xvT, wqC, wkC, wvC, woC, bqv, bkv, out):
    nc = tc.nc

    with (
        tc.tile_pool(name="consts", bufs=1) as consts,
        tc.tile_pool(name="xk_p", bufs=1) as xk_p,
        tc.tile_pool(name="xq_p", bufs=1) as xq_p,
        tc.tile_pool(name="xv_p", bufs=1) as xv_p,
        tc.tile_pool(name="persist", bufs=1) as persist,
        tc.tile_pool(name="attn", bufs=6) as attn_pool,
        tc.tile_pool(name="small", bufs=4) as small,
        tc.tile_pool(name="outs", bufs=3) as outs,
        tc.tile_pool(name="ps_mm", bufs=2, space="PSUM") as ps_mm,
        tc.tile_pool(name="ps_sc", bufs=2, space="PSUM") as ps_sc,
        tc.tile_pool(name="ps_pv", bufs=2, space="PSUM") as ps_pv,
    ):
        # ---------------- constants (DMA-ordered: K weights first) ----------
        wk_s = consts.tile([P, NKC, HC], F16)
        nc.sync.dma_start(wk_s, wkC.rearrange("p (kc m) -> p kc m", kc=NKC))
        bk_s = consts.tile([P, HC // P], F32)
        nc.sync.dma_start(bk_s, bkv.rearrange("(m p) -> p m", p=P))

        ones16 = consts.tile([P, DK], F16)
        nc.vector.memset(ones16, 1.0)
        expb = consts.tile([P, 1], F32)
        nc.vector.memset(expb, EXP_BIAS)

        # ---------------- persistent activations ----------------
        QT = [persist.tile([P, T], F16, name=f"QT{m}") for m in range(2)]
        KT = [persist.tile([P, T], F16, name=f"KT{m}") for m in range(2)]
        V = persist.tile([P, NKB, HL * VW], F16, name="V")
        AC = [persist.tile([P, T], F16, name=f"AC{c}") for c in range(2)]

        # ones columns of V (denominator trick); one cheap ScalarE copy
        nc.scalar.activation(
            V.rearrange("p t (h c) -> p t h c", c=VW)[:, :, :, DK],
            ones16[:, : NKB * HL].rearrange("p (t h) -> p t h", h=HL),
            AF.Copy,
        )

        # x quarter tiles [128, NKC, 512], one batched DMA each (few sync-
        # engine ops; 1KB descriptors).  All stay resident.
        xkt = {}  # tb -> tile
        xqt = {}
        xvt = {}

        def x_load(pool, store, xT, tb, tag, eng=None):
            t = pool.tile([P, NKC, QB], F16, tag=f"{tag}_{tb}", name=f"{tag}_{tb}")
            (eng or nc.sync).dma_start(
                t,
                xT.rearrange("(kc p) t -> p kc t", p=P)[
                    :, :, tb * QB : (tb + 1) * QB
                ],
            )
            store[tb] = t

        # ---------------- projection unit builders ----------------
        def qk_job(store, w_s, b_s, dst, tag, tb, m):
            # one (tb, m) projection job: psum alloc + 8 matmuls + bias evac
            stj = {}
            units = []

            def mk_start(stj=stj):
                stj["ps"] = ps_mm.tile([P, QB], F32, tag="mm", name=f"pj_{tag}{m}{tb}")

            units.append(mk_start)
            for kc in range(NKC):

                def mk_mm(kc=kc, stj=stj):
                    nc.tensor.matmul(
                        stj["ps"],
                        lhsT=w_s[:, kc, m * P : (m + 1) * P],
                        rhs=store[tb][:, kc, :],
                        start=(kc == 0),
                        stop=(kc == NKC - 1),
                    )

                units.append(mk_mm)

            def mk_evac(stj=stj):
                nc.vector.tensor_scalar_add(
                    dst[m][:, tb * QB : (tb + 1) * QB], stj["ps"], b_s[:, m : m + 1]
                )

            units.append(mk_evac)
            return units

        def v_units(tb):
            # V projection for quarter tb: 4 token blocks, single-matmul
            # units (keeps drain bursts small so attention never stalls).
            units = []
            for i in range(QB // P):
                t128 = tb * (QB // P) + i
                stj = {}

                def mk_mm(kc, i=i, stj=stj, tb=tb, t128=t128):
                    if kc == 0:
                        stj["ps"] = ps_mm.tile(
                            [P, HC], F32, tag="mm", name=f"pv_{t128}"
                        )
                    nc.tensor.matmul(
                        stj["ps"],
                        lhsT=xvt[tb][:, kc, i * P : (i + 1) * P],
                        rhs=wv_s[:, kc, :],
                        start=(kc == 0),
                        stop=(kc == NKC - 1),
                    )

                def mk_ev(stj=stj, t128=t128):
                    nc.vector.tensor_copy(
                        V[:, t128].rearrange("p (h c) -> p h c", c=VW)[:, :, :DK],
                        stj["ps"].rearrange("p (h c) -> p h c", c=DK),
                    )

                for kc in range(NKC):
                    units.append(lambda kc=kc, f=mk_mm: f(kc))
                units.append(mk_ev)
            return units

        # ---------------- stage A ----------------
        # Issue all x-tile DMAs up front, ordered by when compute needs them
        # (the DMA engines run far ahead; drain-unit matmuls then never block
        # the in-order PE queue on a just-issued transfer).
        # DMA issue spread across idle engine queues (descriptor generation
        # for a batched x tile costs ~3us of issuing-engine time; the sync
        # queue alone would serialize the prologue).
        wv_s = consts.tile([P, NKC, HC], F16)
        nc.gpsimd.dma_start(wv_s, wvC.rearrange("p (kc m) -> p kc m", kc=NKC))
        wq_s = consts.tile([P, NKC, HC], F16)
        nc.scalar.dma_start(wq_s, wqC.rearrange("p (kc m) -> p kc m", kc=NKC))
        bq_s = consts.tile([P, HC // P], F32)
        nc.scalar.dma_start(bq_s, bqv.rearrange("(m p) -> p m", p=P))
        x_load(xk_p, xkt, xkT, 0, "xk", nc.sync)
        x_load(xv_p, xvt, xvT, 0, "xv", nc.gpsimd)
        x_load(xq_p, xqt, xqT, 0, "xq", nc.scalar)
        x_load(xk_p, xkt, xkT, 1, "xk", nc.sync)
        x_load(xv_p, xvt, xvT, 1, "xv", nc.gpsimd)
        x_load(xq_p, xqt, xqT, 1, "xq", nc.gpsimd)
        x_load(xk_p, xkt, xkT, 2, "xk", nc.sync)
        x_load(xv_p, xvt, xvT, 2, "xv", nc.gpsimd)
        x_load(xq_p, xqt, xqT, 2, "xq", nc.gpsimd)
        x_load(xk_p, xkt, xkT, 3, "xk", nc.sync)
        x_load(xv_p, xvt, xvT, 3, "xv", nc.gpsimd)
        x_load(xq_p, xqt, xqT, 3, "xq", nc.gpsimd)
        wo_s = consts.tile([P, HC // P, D], F16)
        nc.sync.dma_start(wo_s, woC.rearrange("p (c n) -> p c n", c=2))

        for u in qk_job(xkt, wk_s, bk_s, KT, "xk", 0, 0):
            u()
        vq0 = v_units(0)
        for u in vq0[:9]:
            u()  # V block t128-0 (all iteration 0 needs)
        for u in qk_job(xqt, wq_s, bq_s, QT, "xq", 0, 0):
            u()

        # ---------------- zip queue: remaining projections ------------------
        zq = deque()
        zq.extend(vq0[9:])  # V blocks t128 1-3 (needed from iteration 2 on)
        for tb in (1, 2, 3):
            zq.extend(qk_job(xkt, wk_s, bk_s, KT, "xk", tb, 0))
            zq.extend(v_units(tb))
        zq.extend(qk_job(xqt, wq_s, bq_s, QT, "xq", 0, 1))
        for tb in (0, 1, 2, 3):
            zq.extend(qk_job(xkt, wk_s, bk_s, KT, "xk", tb, 1))
        for tb in (1, 2, 3):
            zq.extend(qk_job(xqt, wq_s, bq_s, QT, "xq", tb, 0))
            zq.extend(qk_job(xqt, wq_s, bq_s, QT, "xq", tb, 1))

        urgent = deque()  # deferred normalize / out-proj units (FIFO)

        def drain(n):
            for _ in range(n):
                if urgent:
                    urgent.popleft()()
                elif zq:
                    zq.popleft()()

        # ---------------- deferred normalize / output projection ------------
        def norm_finish(qb, hp, pv0, pv1, last):
            # The pv-PSUM evacuation copies are emitted SYNCHRONOUSLY here
            # (the next head-pair's pv tiles re-bind these PSUM buffers at
            # emission time, so a deferred reader would race).  Only the
            # SBUF-side normalization work is deferred.  On the final group
            # the copies are split across ScalarE (idle, exps done) and DVE.
            m = hp
            st = {}
            # pv's ONLY reader is the pvs copy, so the PSUM bank frees after
            # one DVE op; the partition-0 denominator re-home for gpsimd's
            # broadcast reads the SBUF copy and is deferred off the WAR path.
            for j, (h, pv) in enumerate(((2 * hp, pv0), (2 * hp + 1, pv1))):
                pvs = small.tile([VW, QB], F32, tag="pvs", name=f"pvs_{qb}_{h}")
                if last and j == 1:
                    nc.scalar.copy(pvs, pv)
                else:
                    nc.vector.tensor_copy(pvs, pv)
                st[h] = pvs

            def bc_rcp(h, st=st):
                dn = small.tile([1, QB], F32, tag="dn", name=f"dn_{qb}_{h}")
                nc.vector.tensor_copy(dn, st[h][DK : DK + 1, :])
                db = small.tile([DK, QB], F32, tag="db", name=f"db_{qb}_{h}")
                nc.gpsimd.partition_broadcast(db, dn)
                rb = small.tile([DK, QB], F32, tag="rcp", name=f"rb_{qb}_{h}")
                nc.vector.reciprocal_approx_fast(rb, db)
                st[h, "r"] = rb

            def mul(h, st=st):
                off = 64 * (h % 2)
                nc.vector.tensor_mul(
                    AC[m][off : off + DK, qb * QB : (qb + 1) * QB],
                    st[h][:DK, :],
                    st[h, "r"],
                )

            h0, h1 = 2 * hp, 2 * hp + 1
            return [
                lambda: bc_rcp(h0),
                lambda: mul(h0),
                lambda: bc_rcp(h1),
                lambda: mul(h1),
            ]

        def oproj_units(qb, last):
            units = []
            for i in range(QB // P):
                t128 = qb * (QB // P) + i
                stj = {}

                def mk_mm(n, c, t128=t128, stj=stj):
                    if c == 0:
                        stj[n] = ps_mm.tile(
                            [P, 512], F32, tag="mm", name=f"po_{t128}{n}"
                        )
                    nc.tensor.matmul(
                        stj[n],
                        lhsT=AC[c][:, t128 * P : (t128 + 1) * P],
                        rhs=wo_s[:, c, n * 512 : (n + 1) * 512],
                        start=(c == 0),
                        stop=(c == 1),
                    )

                def mk_ev(n, t128=t128, stj=stj, i=i):
                    if n == 0:
                        stj["ob"] = outs.tile([P, D], F32, tag="ob",
                                              name=f"ob_{t128}")
                    eng = nc.scalar if (last and (i + n) % 2) else nc.vector
                    if eng is nc.scalar:
                        nc.scalar.copy(stj["ob"][:, n * 512 : (n + 1) * 512],
                                       stj[n])
                    else:
                        nc.vector.tensor_copy(
                            stj["ob"][:, n * 512 : (n + 1) * 512], stj[n]
                        )
                    if n == 1:
                        nc.sync.dma_start(
                            out[t128 * P : (t128 + 1) * P, :], stj["ob"]
                        )

                units.append(lambda f=mk_mm: f(0, 0))
                units.append(lambda f=mk_mm: f(0, 1))
                units.append(lambda f=mk_ev: f(0))
                units.append(lambda f=mk_mm: f(1, 0))
                units.append(lambda f=mk_mm: f(1, 1))
                units.append(lambda f=mk_ev: f(1))
            return units

        # ---------------- attention ----------------
        # One flat software-pipelined stream over all (qb, hp) groups: the
        # first score pair of group g+1 is emitted BEFORE the last PV of
        # group g, so the exp stream never waits on a PV->sc turnaround.
        it = [0]

        def drain_n():
            i = it[0]
            it[0] += 1
            if i < 16:
                return 13
            if i < 32:
                return 6
            return 2

        groups = [(qb, hp) for qb in range(NQB) for hp in range(2)]
        prev = None        # (emit_fn,) pending PV pair
        finish = None      # pending group-finish (pv evac + deferred pushes)
        for gi, (qb, hp) in enumerate(groups):
            m = hp  # heads (2*hp, 2*hp+1) live in QT/KT chunk m
            h0, h1 = 2 * hp, 2 * hp + 1
            box = {}

            def mk_emit(kb, at, box=box, h0=h0, h1=h1):
                def emit():
                    nc.tensor.matmul(
                        box["pv0"],
                        lhsT=V[:, kb, VW * h0 : VW * (h0 + 1)],
                        rhs=at[:, :QB],
                        start=(kb == 0),
                        stop=(kb == NKB - 1),
                    )
                    nc.tensor.matmul(
                        box["pv1"],
                        lhsT=V[:, kb, VW * h1 : VW * (h1 + 1)],
                        rhs=at[:, QB:],
                        start=(kb == 0),
                        stop=(kb == NKB - 1),
                    )

                return emit

            for kb in range(NKB):
                sc = ps_sc.tile(
                    [P, 2 * QB], F32, tag="sc", name=f"sc_{qb}_{hp}_{kb}"
                )
                nc.tensor.matmul(
                    sc[:, :QB],
                    lhsT=KT[m][0:DK, kb * P : (kb + 1) * P],
                    rhs=QT[m][0:DK, qb * QB : (qb + 1) * QB],
                    start=True,
                    stop=True,
                )
                nc.tensor.matmul(
                    sc[:, QB:],
                    lhsT=KT[m][DK:P, kb * P : (kb + 1) * P],
                    rhs=QT[m][DK:P, qb * QB : (qb + 1) * QB],
                    start=True,
                    stop=True,
                )
                at = attn_pool.tile(
                    [P, 2 * QB], F16, tag="at", name=f"at_{qb}_{hp}_{kb}"
                )
                nc.scalar.activation(at, sc, AF.Exp, scale=0.125, bias=expb)
                if prev is not None:
                    prev()
                    prev = None
                if finish is not None:
                    finish()
                    finish = None
                if "pv0" not in box:
                    box["pv0"] = ps_pv.tile(
                        [VW, QB], F32, tag="pv", name=f"pv_{qb}_{h0}"
                    )
                    box["pv1"] = ps_pv.tile(
                        [VW, QB], F32, tag="pv", name=f"pv_{qb}_{h1}"
                    )
                prev = mk_emit(kb, at)
                drain(drain_n())

            def mk_finish(qb=qb, hp=hp, box=box, gi=gi):
                def fin():
                    last = gi == len(groups) - 1
                    urgent.extend(
                        norm_finish(qb, hp, box["pv0"], box["pv1"], last)
                    )
                    if hp == 1:
                        urgent.extend(oproj_units(qb, last))

                return fin

            finish = mk_finish()

        prev()
        finish()
        while urgent or zq:
            drain(8)


_module_cache = None


def get_module():
    global _module_cache
    if _module_cache is None:
        _module_cache = build_module()
    return _module_cache


def _chunk_w(wT):
    # [D, HC] -> [128, NKC*HC]: partition-major kc blocks, contiguous rows
    return np.ascontiguousarray(
        wT.reshape(NKC, P, HC).transpose(1, 0, 2).reshape(P, NKC * HC)
    )


def shard_inputs(query, key, value, Wq, bq, Wk, bk, Wv, bv, Wo, bo):
    """Build the 8 per-core input maps (host-side layout/dtype transforms)."""
    f = np.float32
    h = np.float16
    xT = {}
    for b in range(B):
        xT["q", b] = np.ascontiguousarray(np.asarray(query)[:, b, :].T.astype(h))
        xT["k", b] = np.ascontiguousarray(np.asarray(key)[:, b, :].T.astype(h))
        xT["v", b] = np.ascontiguousarray(np.asarray(value)[:, b, :].T.astype(h))
    Wq, Wk, Wv, Wo = (np.asarray(w, f) for w in (Wq, Wk, Wv, Wo))
    bq, bk = np.asarray(bq, f), np.asarray(bk, f)
    in_maps = []
    for c in range(NCORES):
        b, hg = c // (NCORES // B), c % (NCORES // B)
        cols = slice(HC * hg, HC * (hg + 1))
        woT = Wo[:, cols].T.astype(h)  # [HC, D]
        in_maps.append(
            {
                "xqT": xT["q", b],
                "xkT": xT["k", b],
                "xvT": xT["v", b],
                "wqC": _chunk_w(Wq[cols, :].T.astype(h)),
                "wkC": _chunk_w(Wk[cols, :].T.astype(h)),
                "wvC": _chunk_w(Wv[cols, :].T.astype(h)),
                "woC": np.ascontiguousarray(
                    woT.reshape(2, P, D).transpose(1, 0, 2).reshape(P, 2 * D)
                ),
                "bqv": np.ascontiguousarray(bq[cols]),
                "bkv": np.ascontiguousarray(bk[cols]),
            }
        )
    return in_maps


def kernel(query, key, value, Wq, bq, Wk, bk, Wv, bv, Wo, bo, trace=False):
    nc = get_module()
    in_maps = shard_inputs(query, key, value, Wq, bq, Wk, bk, Wv, bv, Wo, bo)
    res = bass_utils.run_bass_kernel_spmd(
        nc, in_maps, core_ids=list(range(NCORES)), trace=trace
    )
    f = np.float32
    bias_term = np.asarray(bv, f) @ np.asarray(Wo, f).T + np.asarray(bo, f)
    output = np.empty((S, B, D), f)
    for b in range(B):
        acc = res.results[4 * b]["out"].astype(f)
        for c in range(4 * b + 1, 4 * b + 4):
            acc = acc + res.results[c]["out"]
        output[:, b, :] = acc + bias_term
    if trace:
        kernel.last_results = res
    return output
